# revision 20
# baseline (speedup 1.0000x reference)
"""DEMONet 3-layer GNN message-passing kernel for 8x Trainium2 NeuronCores.

Math per layer (no deg-0 nodes in this data, bias == 0):
    nm  = segment_sum(h[dst], src) / deg
    out = elu(h @ (Wg + Ws) + nm @ Wl)

Key structure (v2 — dma_gather rewrite):
  * Nodes row-partitioned across 8 cores (12500 real + 300 pad = 12800/core).
    The per-core permutation PRESERVES original-id parity (even id -> even
    slot), so an edge's dst parity is known before any permutation is chosen.
  * Gather table rows PAIR two consecutive permuted nodes: [51200, 128] bf16
    (256B rows, dma_gather's minimum element).  Table = AllGather output
    verbatim (a [NTOT, 64] buffer viewed as [NTOT/2, 128]).
  * Neighbour fetch: nc.gpsimd.dma_gather, ONE instruction per
    (supertile, bucket) gathering SPT*CPG*128 rows (int16 in-bucket pair ids,
    2 buckets of 25600 pair-rows).  This replaces per-128-row
    indirect_dma_start calls whose ~1.2us/call Q7 descriptor generation was
    81% of baseline runtime.
  * Segment-sum: per 128-node tile, 12 accumulating matmuls
    nmP += S_ch^T @ X_ch where S_ch [128e,128n] is a 0/1 indicator built on
    DVE.  S is built with ALL-packed-last-dim operands (lsrc duplicated in
    pairs host-side; iota viewed [64,2]) to qualify for the DVE 2x/4x
    perf modes; a plain broadcast is_equal runs 1x and would dominate.
  * Weight matmuls produce the TRANSPOSED output
    outT[64o,128n] = Wgs^T@hT + Wl^T@nmT, so the next layer's h^T operand
    (SBUF-resident hT_all [64, 12800] bf16, no DRAM round trip) comes out of
    ELU directly.  Row-major table rows are made with small PE transposes.
  * ELU(x) = max(x, min(exp(x),1)-1): one ACT exp + two DVE ops per 2 tiles.
"""

import os
import numpy as np
import ml_dtypes

import concourse.bass as bass
import concourse.bacc as bacc
import concourse.mybir as mybir
import concourse.tile as tile
from concourse.bass_utils import run_bass_kernel_spmd
from concourse.masks import make_identity
from concourse import library_config

F32 = mybir.dt.float32
BF16 = mybir.dt.bfloat16
I16 = mybir.dt.int16
BF_NP = ml_dtypes.bfloat16

P = 128   # partitions / tile node count / chunk edge count
D = 64    # feature dim


class Cfg:
    def __init__(self, n_nodes=100000, n_cores=8, npc_raw=12500, npc=12800,
                 spt=10, ctb=3, n_layers=3):
        self.n_nodes = n_nodes
        self.n_cores = n_cores
        self.npc_raw = npc_raw
        self.npc = npc
        self.tpc = npc // P                  # tiles per core (100)
        self.spt = spt                       # tiles per supertile
        assert self.tpc % spt == 0
        self.nst = self.tpc // spt
        self.nb = 2                          # pair-row buckets
        self.par = 2                         # dst parity groups
        self.ctb = ctb                       # chunks per (tile,bucket,parity)
        self.cpg = self.par * ctb            # chunks per (tile,bucket)
        self.cpt = self.nb * self.cpg        # chunks per tile (12)
        self.j = spt * self.cpt              # chunks per supertile
        self.ntot = n_cores * npc
        self.npair = self.ntot // 2
        self.bs = self.npair // self.nb      # pair rows per bucket (25600)
        assert self.bs <= 32767              # int16 dma_gather indices
        self.gi = spt * self.cpg * P         # gathered rows per bucket region
        # dma_gather call plan within a bucket region: (chunk0, nchunks).
        # One call per (supertile, bucket): 7680 idxs -> 481 descs per DMA
        # engine, well under the 1024-desc carveout ring; Q7 desc-gen fixed
        # overhead is paid once instead of 8x.
        plan = os.environ.get("GNN_CALL_PLAN", "8")
        sizes = [int(t) for t in plan.split(",")]
        self.calls = []
        c0 = 0
        i = 0
        rch = spt * self.cpg
        while c0 < rch:
            n = min(sizes[i % len(sizes)], rch - c0)
            self.calls.append((c0, n))
            c0 += n
            i += 1
        self.n_layers = n_layers


def _pack_core(cnt4, node_par, node_half, tpc, cap):
    """Assign nodes to tpc tiles of 128 slots (64 even-parity + 64 odd,
    half-h nodes only in tiles [h*tpc/2, (h+1)*tpc/2)) s.t. per-tile
    per-(bucket,parity) edge counts <= cap.  Returns slot within core."""
    n = cnt4.shape[0]
    half_mask = np.zeros((2, tpc), bool)
    half_mask[0, :tpc // 2] = True
    half_mask[1, tpc // 2:] = True
    order = np.argsort(-cnt4.sum(1), kind="stable")
    rem = np.full((tpc, 4), cap, np.int64)
    slots = np.full((tpc, 2), P // 2, np.int64)
    tile_of = np.full(n, -1, np.int32)
    for i in order:
        s = cnt4[i]
        pr = node_par[i]
        cand = (half_mask[node_half[i]] & (slots[:, pr] > 0)
                & np.all(rem >= s, axis=1))
        if not cand.any():
            raise RuntimeError("node packing failed; loosen ctb")
        score = (rem - s).min(1).astype(np.float64) + 1e-3 * slots.sum(1)
        score[~cand] = -1e18
        t = int(np.argmax(score))
        rem[t] -= s
        slots[t, pr] -= 1
        tile_of[i] = t
    # slot within tile: parity-interleaved, stable placement order
    slot_of = np.empty(n, np.int64)
    for pr in (0, 1):
        m = node_par == pr
        key = tile_of[m].astype(np.int64)
        o = np.argsort(key, kind="stable")
        within = np.arange(m.sum()) - np.searchsorted(key[o], key[o])
        sl = np.empty(m.sum(), np.int64)
        sl[o] = key[o] * P + 2 * within + pr
        slot_of[m] = sl
    return slot_of


def prep_host(x, edge_index, cfg: Cfg):
    N, NC, NPC_RAW, NPC, TPC = (cfg.n_nodes, cfg.n_cores, cfg.npc_raw,
                                cfg.npc, cfg.tpc)
    SPT, NST, NB, CTB, CPG, CPT, J, GI, BS = (
        cfg.spt, cfg.nst, cfg.nb, cfg.ctb, cfg.cpg, cfg.cpt, cfg.j,
        cfg.gi, cfg.bs)
    src = np.asarray(edge_index[0], dtype=np.int64)
    dst = np.asarray(edge_index[1], dtype=np.int64)
    E = src.shape[0]

    deg = np.bincount(src, minlength=N)
    if deg.min() == 0:
        raise NotImplementedError("deg-0 nodes present; Wg+Ws fusion invalid")
    inv_deg = (1.0 / deg).astype(np.float32)

    c_src = np.minimum(src // NPC_RAW, NC - 1)
    c_dst = np.minimum(dst // NPC_RAW, NC - 1)
    # bucket = which HALF of the owner core's tiles dst lands in; preserved
    # statically by the permutation (like parity) so the per-half AllGather
    # can overlap with compute.
    bucket = ((dst >> 1) & 1).astype(np.int64)
    par = (dst % 2).astype(np.int64)       # original-id parity (preserved)

    # per-node out-degree per (bucket,parity), then pack nodes into tiles
    key4 = (bucket * 2 + par).astype(np.int64)
    cnt4 = np.zeros((N, 4), np.int32)
    np.add.at(cnt4, (src, key4), 1)
    perm_pos = np.zeros(N, np.int64)       # orig id -> slot within core
    for c in range(NC):
        lo, hi = c * NPC_RAW, min((c + 1) * NPC_RAW, N)
        ids = np.arange(lo, hi)
        perm_pos[lo:hi] = _pack_core(cnt4[lo:hi], (ids % 2).astype(np.int64),
                                     ((ids >> 1) & 1).astype(np.int64),
                                     TPC, CTB * P)
    gpos = np.minimum(np.arange(N) // NPC_RAW, NC - 1) * NPC + perm_pos

    # edge -> (core, supertile, chunk, partition) slots
    half_npc = NPC // 2
    pp_dst = perm_pos[dst]
    # bucket-A table is core-major [core][row]; bucket-B is PIECE-major
    # [piece][core][row] so the 3 partial AllGathers write contiguous
    # slices (pieces: within-half rows [0,3B), [3B,4B), [4B,5B),
    # B = SPT*P rows).
    stp = (NPC // TPC) * cfg.spt              # SPT*P rows
    b_plo = np.array([0, 3 * stp, 4 * stp], np.int64)
    b_phi = np.array([3 * stp, 4 * stp, 5 * stp], np.int64)
    b_base = np.concatenate([[0], np.cumsum(NC * (b_phi - b_plo))])[:3]
    r_half = pp_dst % half_npc
    piece = np.searchsorted(b_phi, r_half, side="right")
    rows_p = (b_phi - b_plo)[piece]
    rowB = b_base[piece] + np.minimum(dst // NPC_RAW, NC - 1) * rows_p \
        + (r_half - b_plo[piece])
    pairA = (np.minimum(dst // NPC_RAW, NC - 1) * (half_npc // 2)
             + r_half // 2)
    pair_in_bucket = np.where(bucket == 0, pairA, rowB >> 1)
    lsrc_val = (perm_pos[src] % P).astype(np.int16)
    tile_of_src = perm_pos[src] // P
    s_of = tile_of_src // SPT
    t_in = tile_of_src % SPT
    grp = ((((c_src * NST + s_of) * NB + bucket) * SPT + t_in)
           * 2 + par)
    n_groups = NC * NST * NB * SPT * 2
    counts = np.bincount(grp, minlength=n_groups)
    assert counts.max() <= CTB * P, (counts.max(), CTB * P)
    order = np.argsort(grp, kind="stable")
    starts = np.zeros(n_groups + 1, np.int64)
    np.cumsum(counts, out=starts[1:])
    q = np.arange(E) - starts[grp[order]]
    ks = grp[order]
    g_c = ks // (NST * NB * SPT * 2)
    g_s = (ks // (NB * SPT * 2)) % NST
    g_b = (ks // (SPT * 2)) % NB
    g_t = (ks // 2) % SPT
    g_p = ks % 2
    kk = q // P
    p_slot = q % P
    chunk = g_b * (SPT * CPG) + g_t * CPG + g_p * CTB + kk   # [0, J)
    c_loc = chunk - g_b * (SPT * CPG)                        # [0, J/NB)
    slot_i = c_loc * P + p_slot                              # [0, GI)

    idx_lin = np.zeros((NC, NST, NB, GI), np.int16)
    idx_lin[g_c, g_s, g_b, slot_i] = pair_in_bucket[order].astype(np.int16)
    # dma_gather is limited to 1024 indices per call on this runtime, so each
    # (supertile, bucket) region of GI slots is gathered by a sequence of
    # calls (cfg.calls).  Each call's index segment is independently wrapped:
    # wrapped[p, s16] = seg[s16*16 + p]; partitions replicated x8; calls and
    # buckets concatenated along the free dim.
    segs = []
    for b in range(NB):
        o = 0
        for (c0, nch) in cfg.calls:
            n = nch * P
            seg = idx_lin[..., b, o:o + n]                    # [NC,NST,n]
            segs.append(seg.reshape(NC, NST, n // 16, 16)
                        .transpose(0, 1, 3, 2))               # [NC,NST,16,n/16]
            o += n
        assert o == GI
    wrapped = np.concatenate(segs, axis=3)                    # [NC,NST,16,NB*GI/16]
    gidx16 = np.ascontiguousarray(np.tile(wrapped, (1, 1, 8, 1)))

    lsrc2 = np.full((NC, NST, P, J, 2), 300, np.int16)
    lsrc2[g_c, g_s, p_slot, chunk, 0] = lsrc_val[order]
    lsrc2[g_c, g_s, p_slot, chunk, 1] = lsrc_val[order]
    lsrc2 = np.ascontiguousarray(lsrc2.reshape(NC, NST, P, 2 * J))

    x = np.asarray(x, dtype=np.float32)
    x_pad = np.zeros((cfg.ntot, D), np.float32)
    invdeg_pad = np.zeros(cfg.ntot, np.float32)
    x_pad[gpos] = x
    invdeg_pad[gpos] = inv_deg
    x3 = x_pad.reshape(NC, 2, NPC // 2, D).astype(BF_NP)
    xfA = np.ascontiguousarray(x3[:, 0].reshape(cfg.ntot // 2, D))
    xfB = np.ascontiguousarray(np.concatenate(
        [x3[:, 1, lo:hi, :].reshape(-1, D)
         for lo, hi in zip(b_plo, b_phi)], axis=0))

    per_core = []
    for c in range(NC):
        xs = x_pad[c * NPC:(c + 1) * NPC]
        per_core.append(dict(
            x_hT=np.ascontiguousarray(xs.T.astype(BF_NP)),        # [64, NPC]
            xfA=xfA, xfB=xfB,                                     # [NTOT/2, 64]
            gidx16=np.ascontiguousarray(gidx16[c]),               # [NST,128,2*GI/16]
            lsrc2=np.ascontiguousarray(lsrc2[c]),                 # [NST,128,2J]
            invdegT=np.ascontiguousarray(
                invdeg_pad[c * NPC:(c + 1) * NPC].reshape(TPC, P).T),
        ))
    return per_core, perm_pos


def build_program(nc, cfg: Cfg):
    NPC, NTOT, NST, J, TPC, GI, NB, BS = (
        cfg.npc, cfg.ntot, cfg.nst, cfg.j, cfg.tpc, cfg.gi, cfg.nb, cfg.bs)
    NL = cfg.n_layers

    x_hT = nc.dram_tensor("x_hT", [D, NPC], BF16, kind="ExternalInput")
    xfA = nc.dram_tensor("xfA", [NTOT // 2, D], BF16, kind="ExternalInput")
    xfB = nc.dram_tensor("xfB", [NTOT // 2, D], BF16, kind="ExternalInput")
    gidx16 = nc.dram_tensor("gidx16", [NST, P, NB * (GI // 16)], I16,
                            kind="ExternalInput")
    lsrc2 = nc.dram_tensor("lsrc2", [NST, P, 2 * J], I16, kind="ExternalInput")
    invdegT = nc.dram_tensor("invdegT", [P, TPC], F32, kind="ExternalInput")
    w_gs = [nc.dram_tensor(f"w_gs{L}", [D, D], BF16, kind="ExternalInput")
            for L in range(NL)]
    w_l = [nc.dram_tensor(f"w_l{L}", [D, D], BF16, kind="ExternalInput")
           for L in range(NL)]
    w_c = [nc.dram_tensor(f"w_c{L}", [2, D], BF16, kind="ExternalInput")
           for L in range(NL)]
    out_own = nc.dram_tensor("out_own", [NPC, D], BF16, kind="ExternalOutput")

    ag_in = [nc.dram_tensor(f"ag_in{L}", [NPC, D], BF16, kind="Internal")
             for L in range(NL - 1)]
    shared = "Shared" if cfg.n_cores > 4 else "Local"
    h_fullA = [nc.dram_tensor(f"h_fullA{L}", [NTOT // 2, D], BF16,
                              kind="Internal", addr_space=shared)
               for L in range(NL - 1)]
    h_fullB = [nc.dram_tensor(f"h_fullB{L}", [NTOT // 2, D], BF16,
                              kind="Internal", addr_space=shared)
               for L in range(NL - 1)]

    with tile.TileContext(nc) as tc:
        _emit(nc, tc, cfg, locals())
    return nc


def _emit(nc, tc, cfg: Cfg, T):
    NPC, NTOT, NST, SPT, NB, CTB, CPG, J, TPC, GI, BS, NL = (
        cfg.npc, cfg.ntot, cfg.nst, cfg.spt, cfg.nb, cfg.ctb, cfg.cpg,
        cfg.j, cfg.tpc, cfg.gi, cfg.bs, cfg.n_layers)
    x_hT, xfA, xfB, gidx16, lsrc2, invdegT = (
        T["x_hT"], T["xfA"], T["xfB"], T["gidx16"], T["lsrc2"], T["invdegT"])
    w_gs, w_l, w_c, out_own = T["w_gs"], T["w_l"], T["w_c"], T["out_own"]
    ag_in, h_fullA, h_fullB = T["ag_in"], T["h_fullA"], T["h_fullB"]
    RCH = SPT * CPG               # chunks per bucket region within supertile
    # tile groups per supertile for the batched weight matmuls / ELU
    GROUPS = [(0, 4), (4, 4), (8, 2)]

    with (
        tc.tile_pool(name="const", bufs=1) as constp,
        tc.tile_pool(name="io", bufs=5) as iop,
        tc.tile_pool(name="bigX", bufs=3) as bigx,
        tc.tile_pool(name="bigS", bufs=3) as bigs,
        tc.tile_pool(name="small", bufs=3) as smallp,
        tc.tile_pool(name="grp", bufs=3) as grpp,
        tc.tile_pool(name="rows", bufs=3) as rowsp,
        tc.tile_pool(name="psA", bufs=3, space="PSUM") as psA,
        tc.tile_pool(name="psB", bufs=1, space="PSUM") as psB,
        tc.tile_pool(name="psC", bufs=2, space="PSUM") as psC,
        tc.tile_pool(name="psD", bufs=2, space="PSUM") as psD,
    ):
        identF = constp.tile([P, P], F32, name="identF")
        make_identity(nc, identF[:])
        iota16 = constp.tile([P, P], I16, name="iota16")
        nc.gpsimd.iota(iota16[:], pattern=[[1, P]], base=0,
                       channel_multiplier=0)
        invdeg_sb = constp.tile([P, TPC], F32, name="invdeg_sb")
        nc.sync.dma_start(invdeg_sb[:], invdegT[:])
        wgs_sb, wl_sb, wc_sb = [], [], []
        for L in range(NL):
            wg_t = constp.tile([D, D], BF16, name=f"wgs_sb{L}")
            nc.sync.dma_start(wg_t[:], w_gs[L][:])
            wgs_sb.append(wg_t)
            wl_t = constp.tile([D, D], BF16, name=f"wl_sb{L}")
            nc.sync.dma_start(wl_t[:], w_l[L][:])
            wl_sb.append(wl_t)
            wc_t = constp.tile([2, D], BF16, name=f"wc_sb{L}")
            nc.sync.dma_start(wc_t[:], w_c[L][:])
            wc_sb.append(wc_t)
        ones2 = constp.tile([2, SPT * P], BF16, name="ones2")
        nc.vector.memset(ones2[:], 1.0)
        negone = constp.tile([D, 1], F32, name="negone")
        nc.vector.memset(negone[:], -1.0)
        ones_f = constp.tile([D, 4 * P], F32, name="ones_f")
        nc.vector.memset(ones_f[:], 1.0)
        # layer-input h^T, SBUF-resident across the whole kernel
        hT_all = constp.tile([D, NPC], BF16, name="hT_all")
        nc.sync.dma_start(hT_all[:], x_hT[:])
        r_full = nc.gpsimd.to_reg(8 * P)
        r_regs = {8 * P: r_full}
        # per-queue DMA completion sems for PREPARE_ONLY gathers: desc-gen
        # runs ahead of data deps (which defer to the trigger), so the Q7
        # feeder never stalls on buffer/collective waits mid-stream.
        gat_sems = [nc.alloc_semaphore(f"gat_dma_q{q}") for q in range(4)]

        def emit_gather(L, s, X3, idx_t, tables, bsel):
            qn = (s * NB + bsel) * len(cfg.calls)
            for (c0, nch) in cfg.calls:
                col0 = bsel * (GI // 16) + c0 * (P // 16)
                n = nch * P
                if n not in r_regs:
                    r_regs[n] = nc.gpsimd.to_reg(n)
                nc.gpsimd.dma_gather(
                    X3[:, bsel * RCH + c0:bsel * RCH + c0 + nch, :],
                    tables[bsel],
                    idx_t[:, col0:col0 + nch * (P // 16)],
                    n, r_regs[n], P, elem_step=P,
                    single_packet=(n <= 1024),
                    queue_num=qn % 4)
                qn += 1

        for L in range(NL):
            last = L == NL - 1
            tabs = ((xfA, xfB) if L == 0
                    else (h_fullA[L - 1], h_fullB[L - 1]))
            tables = [t[:].rearrange("(a b) d -> a (b d)", b=2)
                      for t in tabs]                # [NPAIR/2, 128] views

            def compute_st(s, X3, ls2_t, L=L, last=last):
                ls3 = ls2_t[:].rearrange("p (c b) -> p c b", b=2)
                io3 = iota16[:].rearrange("p (a b) -> p a b", b=2)
                Sp = {}

                def s_build(pp):
                    # indicator chunks for tile pair pp (tiles 2pp, 2pp+1):
                    # pair tile [128, 2*CPT chunks, 128], one op per bucket
                    # block; all operands packed-last-dim -> DVE 2x mode
                    St = bigs.tile([P, 2 * cfg.cpt * P], BF16, tag="Sp",
                                   name=f"S_{L}_{s}_{pp}")
                    Sp[pp] = St[:].rearrange("p (c e) -> p c e", e=P)
                    S4 = St[:].rearrange("p (c a b) -> p c a b", a=64, b=2)
                    w = 2 * CPG
                    for b in range(NB):
                        lo = b * RCH + 2 * pp * CPG
                        nc.vector.tensor_tensor(
                            out=S4[:, b * w:(b + 1) * w],
                            in0=ls3[:, lo:lo + w, None, :]
                                .to_broadcast([P, w, 64, 2]),
                            in1=io3[:, None, :, :]
                                .to_broadcast([P, w, 64, 2]),
                            op=mybir.AluOpType.is_equal,
                        )

                rows_st = rowsp.tile([P, SPT * D], BF16, tag="rows",
                                     name=f"rw_{L}_{s}")
                nmPs = [None] * SPT
                outPs = {}
                nmTg = {}
                h32g = {}

                def scatter_pair(ta, tb):
                    pa = psA.tile([P, D], F32, tag="nmP",
                                  name=f"nmP_{L}_{s}_{ta}")
                    pb = psA.tile([P, D], F32, tag="nmP",
                                  name=f"nmP_{L}_{s}_{tb}")
                    nmPs[ta], nmPs[tb] = pa, pb
                    St = Sp.pop(ta // 2)
                    ci = 0
                    for b in range(NB):
                        for pr in range(2):
                            for k in range(CTB):
                                for tt, pp in ((0, pa), (1, pb)):
                                    ch = b * RCH + (ta + tt) * CPG \
                                        + pr * CTB + k
                                    sc = b * 2 * CPG + tt * CPG \
                                        + pr * CTB + k
                                    nc.tensor.matmul(
                                        pp[:],
                                        lhsT=St[:, sc, :],
                                        rhs=X3[:, ch, pr * D:(pr + 1) * D],
                                        start=(ci == 0),
                                        stop=(ci == cfg.cpt - 1),
                                    )
                                ci += 1

                nmTPs = {}

                def tail(t, g, gw):
                    g_t = s * SPT + t
                    nm_sb = smallp.tile([P, D], F32, tag="nm",
                                        name=f"nm_{L}_{s}_{t}")
                    nc.vector.tensor_scalar_mul(
                        nm_sb[:], nmPs[t][:], invdeg_sb[:, g_t:g_t + 1])
                    if t % 2 == 0:
                        nmTPs[t // 2] = psB.tile([D, 2 * P], F32, tag="nmTP",
                                                 name=f"nmTP_{L}_{s}_{t}")
                    nmTP = nmTPs[t // 2][:, (t % 2) * P:(t % 2 + 1) * P]
                    nc.tensor.transpose(nmTP, nm_sb[:], identF[:])
                    if g not in nmTg:
                        nmTg[g] = grpp.tile([D, 4 * P], BF16, tag="nmTg",
                                            name=f"nmTg_{L}_{s}_{g}")
                    off = (t - GROUPS[g][0]) * P
                    nc.scalar.copy(nmTg[g][:, off:off + P], nmTP)

                def group_ops(g):
                    t0, ntg = GROUPS[g]
                    W = ntg * P
                    c0 = s * SPT + t0
                    outP = psC.tile([D, 4 * P], F32, tag="outP",
                                    name=f"oP_{L}_{s}_{g}")
                    outPs[g] = outP
                    reg = outP[:, :W]
                    nc.tensor.matmul(reg, lhsT=wgs_sb[L][:],
                                     rhs=hT_all[:, c0 * P:c0 * P + W],
                                     start=True, stop=False)
                    nc.tensor.matmul(reg, lhsT=wl_sb[L][:],
                                     rhs=nmTg[g][:, :W],
                                     start=False, stop=False)
                    nc.tensor.matmul(reg, lhsT=wc_sb[L][:],
                                     rhs=ones2[:, :W],
                                     start=False, stop=True)
                    # ELU'+1: h' = max(z+1, min(exp(z), 1)); outP holds z+1
                    e_sb = smallp.tile([D, 4 * P], F32, tag="elu_e",
                                       name=f"e_{L}_{s}_{g}")
                    nc.scalar.activation(e_sb[:, :W], reg,
                                         mybir.ActivationFunctionType.Exp,
                                         bias=negone[:])
                    c_sb = smallp.tile([D, 4 * P], F32, tag="elu_c",
                                       name=f"c_{L}_{s}_{g}")
                    nc.vector.tensor_tensor(out=c_sb[:, :W], in0=e_sb[:, :W],
                                            in1=ones_f[:, :W],
                                            op=mybir.AluOpType.min)
                    h32 = grpp.tile([D, 4 * P], F32, tag="h32",
                                    name=f"h32_{L}_{s}_{g}")
                    h32g[g] = h32
                    nc.vector.tensor_tensor(out=h32[:, :W], in0=c_sb[:, :W],
                                            in1=reg, op=mybir.AluOpType.max)
                    if not last:
                        nc.scalar.copy(hT_all[:, c0 * P:c0 * P + W],
                                       h32[:, :W])
                    # row-major rows via PE transposes + one DVE evac
                    rPg = psD.tile([P, 4 * D], F32, tag="rPg",
                                   name=f"rPg_{L}_{s}_{g}")
                    for t in range(t0, t0 + ntg):
                        nc.tensor.transpose(
                            rPg[:, (t - t0) * D:(t - t0 + 1) * D],
                            h32[:, (t - t0) * P:(t - t0 + 1) * P],
                            identF[:D, :D])
                    if last:
                        nc.vector.tensor_scalar_add(
                            rows_st[:, t0 * D:(t0 + ntg) * D],
                            rPg[:, :ntg * D], -1.0)
                    else:
                        nc.vector.tensor_copy(
                            rows_st[:, t0 * D:(t0 + ntg) * D],
                            rPg[:, :ntg * D])

                grp_of_t = {}
                for g, (t0g, ntg) in enumerate(GROUPS):
                    for t in range(t0g, t0g + ntg):
                        grp_of_t[t] = g
                npair = SPT // 2
                s_build(0)
                for p in range(npair + 1):
                    if p < npair:
                        if p + 1 < npair:
                            s_build(p + 1)
                        scatter_pair(2 * p, 2 * p + 1)
                    if p >= 1:
                        for t in (2 * (p - 1), 2 * (p - 1) + 1):
                            g = grp_of_t[t]
                            tail(t, g, None)
                            if t == GROUPS[g][0] + GROUPS[g][1] - 1:
                                group_ops(g)

                dst_rows = (out_own if last else ag_in[L]).rearrange(
                    "(s t p) d -> s p t d", s=NST, t=SPT, p=P)
                nc.sync.dma_start(
                    dst_rows[s],
                    rows_st[:].rearrange("p (t d) -> p t d", d=D))

            def emit_ag(cs, L=L, last=last):
                # AllGathers fire as soon as the rows they need are written.
                # bucket-B table (piece-major layout) gathered in 3 slices so
                # only the last supertile's small tail is exposed at the
                # layer boundary.
                if last:
                    return
                stp = SPT * P
                if cs == NST // 2 - 1:
                    nc.gpsimd.collective_compute(
                        "AllGather",
                        mybir.AluOpType.bypass,
                        replica_groups=[list(range(cfg.n_cores))],
                        ins=[ag_in[L][0:NPC // 2, :]],
                        outs=[h_fullA[L][:]],
                    )
                if cs in (NST - 3, NST - 2):
                    r0 = 0 if cs == NST - 3 else 3 * stp
                    r1 = 3 * stp if cs == NST - 3 else 4 * stp
                    nc.gpsimd.collective_compute(
                        "AllGather",
                        mybir.AluOpType.bypass,
                        replica_groups=[list(range(cfg.n_cores))],
                        ins=[ag_in[L][NPC // 2 + r0:NPC // 2 + r1, :]],
                        outs=[h_fullB[L][cfg.n_cores * r0:
                                         cfg.n_cores * r1, :]],
                    )
                if cs == NST - 1:
                    nc.gpsimd.collective_compute(
                        "AllGather",
                        mybir.AluOpType.bypass,
                        replica_groups=[list(range(cfg.n_cores))],
                        ins=[ag_in[L][NPC - stp:NPC, :]],
                        outs=[h_fullB[L][cfg.n_cores * 4 * stp:, :]],
                    )

            for s in range(NST):
                idx_t = iop.tile([P, NB * (GI // 16)], I16, tag="idx",
                                 name=f"idx_{L}_{s}")
                nc.sync.dma_start(idx_t[:], gidx16[s, :, :])
                ls2_t = iop.tile([P, 2 * J], I16, tag="ls2",
                                 name=f"ls2_{L}_{s}")
                nc.sync.dma_start(ls2_t[:], lsrc2[s, :, :])
                X = bigx.tile([P, J * P], BF16, tag="X", name=f"X_{L}_{s}")
                X3 = X[:].rearrange("p (c e) -> p c e", e=P)
                emit_gather(L, s, X3, idx_t, tables, 0)
                emit_gather(L, s, X3, idx_t, tables, 1)
                compute_st(s, X3, ls2_t)
                emit_ag(s)


def kernel(**inputs):
    cfg = Cfg()
    x = np.asarray(inputs["x"], np.float32)
    ei = np.asarray(inputs["edge_index"])
    Ws = []
    for L, (a, b, c, bias) in enumerate(
            [("Wg1", "Wl1", "Ws1", "b1"), ("Wg2", "Wl2", "Ws2", "b2"),
             ("Wgo", "Wlo", "Wso", "bo")]):
        bv = np.asarray(inputs[bias], np.float32)
        assert np.all(bv == 0.0), "nonzero bias not supported by this build"
        wgs = (np.asarray(inputs[a], np.float32)
               + np.asarray(inputs[c], np.float32))
        wl = np.asarray(inputs[b], np.float32)
        # const row for the z+1 shift: layer 0 inputs are unshifted (raw x),
        # later layers consume h' = h+1, so subtract the weight column sums.
        cvec = np.ones(D, np.float32)
        if L > 0:
            cvec = cvec - wgs.sum(0) - wl.sum(0)
        c_hi = cvec.astype(BF_NP)
        c_lo = (cvec - c_hi.astype(np.float32)).astype(BF_NP)
        Ws.append((wgs.astype(BF_NP), wl.astype(BF_NP),
                   np.ascontiguousarray(np.stack([c_hi, c_lo]))))

    per_core, perm_pos = prep_host(x, ei, cfg)

    nc = bacc.Bacc("TRN2", target_bir_lowering=False, debug=False,
                   enable_asserts=False, num_devices=cfg.n_cores,
                   num_swdge_queues=4)
    build_program(nc, cfg)
    nc.compile()

    in_maps = []
    for c in range(cfg.n_cores):
        m = dict(per_core[c])
        for L in range(3):
            m[f"w_gs{L}"] = Ws[L][0]
            m[f"w_l{L}"] = Ws[L][1]
            m[f"w_c{L}"] = Ws[L][2]
        in_maps.append(m)

    res = run_bass_kernel_spmd(
        nc, in_maps, core_ids=list(range(cfg.n_cores)),
        trace=bool(int(os.environ.get("GNN_TRACE", "0"))),
    )
    full = np.zeros((cfg.n_nodes, D), np.float32)
    for c in range(cfg.n_cores):
        lo = c * cfg.npc_raw
        hi = min((c + 1) * cfg.npc_raw, cfg.n_nodes)
        full[lo:hi] = res.results[c]["out_own"].astype(
            np.float32)[perm_pos[lo:hi]]
    kernel.last_results = res
    return full.astype(np.float32)



# revision 21
# speedup vs baseline: 1.1868x; 1.1868x over previous
"""DEMONet 3-layer GNN message-passing kernel for 8x Trainium2 NeuronCores.

Math per layer (no deg-0 nodes in this data, bias == 0):
    nm  = segment_sum(h[dst], src) / deg
    out = elu(h @ (Wg + Ws) + nm @ Wl)

Key structure (v2 — dma_gather rewrite):
  * Nodes row-partitioned across 8 cores (12500 real + 300 pad = 12800/core).
    The per-core permutation PRESERVES original-id parity (even id -> even
    slot), so an edge's dst parity is known before any permutation is chosen.
  * Gather table rows PAIR two consecutive permuted nodes: [51200, 128] bf16
    (256B rows, dma_gather's minimum element).  Table = AllGather output
    verbatim (a [NTOT, 64] buffer viewed as [NTOT/2, 128]).
  * Neighbour fetch: nc.gpsimd.dma_gather, ONE instruction per
    (supertile, bucket) gathering SPT*CPG*128 rows (int16 in-bucket pair ids,
    2 buckets of 25600 pair-rows).  This replaces per-128-row
    indirect_dma_start calls whose ~1.2us/call Q7 descriptor generation was
    81% of baseline runtime.
  * Segment-sum: per 128-node tile, 12 accumulating matmuls
    nmP += S_ch^T @ X_ch where S_ch [128e,128n] is a 0/1 indicator built on
    DVE.  S is built with ALL-packed-last-dim operands (lsrc duplicated in
    pairs host-side; iota viewed [64,2]) to qualify for the DVE 2x/4x
    perf modes; a plain broadcast is_equal runs 1x and would dominate.
  * Weight matmuls produce the TRANSPOSED output
    outT[64o,128n] = Wgs^T@hT + Wl^T@nmT, so the next layer's h^T operand
    (SBUF-resident hT_all [64, 12800] bf16, no DRAM round trip) comes out of
    ELU directly.  Row-major table rows are made with small PE transposes.
  * ELU(x) = max(x, min(exp(x),1)-1): one ACT exp + two DVE ops per 2 tiles.
"""

import os
import numpy as np
import ml_dtypes

import concourse.bass as bass
import concourse.bacc as bacc
import concourse.mybir as mybir
import concourse.tile as tile
from concourse.bass_utils import run_bass_kernel_spmd
from concourse.masks import make_identity
from concourse import library_config

F32 = mybir.dt.float32
BF16 = mybir.dt.bfloat16
I16 = mybir.dt.int16
BF_NP = ml_dtypes.bfloat16

P = 128   # partitions / tile node count / chunk edge count
D = 64    # feature dim


class Cfg:
    def __init__(self, n_nodes=100000, n_cores=8, npc_raw=12500, npc=12800,
                 spt=10, ctb=3, n_layers=3):
        self.n_nodes = n_nodes
        self.n_cores = n_cores
        self.npc_raw = npc_raw
        self.npc = npc
        self.tpc = npc // P                  # tiles per core (100)
        self.spt = spt                       # tiles per supertile
        assert self.tpc % spt == 0
        self.nst = self.tpc // spt
        self.nb = 2                          # pair-row buckets
        self.par = 2                         # dst parity groups
        self.ctb = ctb                       # chunks per (tile,bucket,parity)
        self.cpg = self.par * ctb            # chunks per (tile,bucket)
        self.cpt = self.nb * self.cpg        # chunks per tile (12)
        self.j = spt * self.cpt              # chunks per supertile
        self.ntot = n_cores * npc
        self.npair = self.ntot // 2
        self.bs = self.npair // self.nb      # pair rows per bucket (25600)
        assert self.bs <= 32767              # int16 dma_gather indices
        self.gi = spt * self.cpg * P         # gathered rows per bucket region
        # dma_gather call plan within a bucket region: (chunk0, nchunks).
        # One call per (supertile, bucket): 7680 idxs -> 481 descs per DMA
        # engine, well under the 1024-desc carveout ring; Q7 desc-gen fixed
        # overhead is paid once instead of 8x.
        plan = os.environ.get("GNN_CALL_PLAN", "8")
        sizes = [int(t) for t in plan.split(",")]
        self.calls = []
        c0 = 0
        i = 0
        rch = spt * self.cpg
        while c0 < rch:
            n = min(sizes[i % len(sizes)], rch - c0)
            self.calls.append((c0, n))
            c0 += n
            i += 1
        self.n_layers = n_layers


def _pack_core(cnt4, node_par, node_half, tpc, cap):
    """Assign nodes to tpc tiles of 128 slots (64 even-parity + 64 odd,
    half-h nodes only in tiles [h*tpc/2, (h+1)*tpc/2)) s.t. per-tile
    per-(bucket,parity) edge counts <= cap.  Returns slot within core."""
    n = cnt4.shape[0]
    half_mask = np.zeros((2, tpc), bool)
    half_mask[0, :tpc // 2] = True
    half_mask[1, tpc // 2:] = True
    order = np.argsort(-cnt4.sum(1), kind="stable")
    rem = np.full((tpc, 4), cap, np.int64)
    slots = np.full((tpc, 2), P // 2, np.int64)
    tile_of = np.full(n, -1, np.int32)
    for i in order:
        s = cnt4[i]
        pr = node_par[i]
        cand = (half_mask[node_half[i]] & (slots[:, pr] > 0)
                & np.all(rem >= s, axis=1))
        if not cand.any():
            raise RuntimeError("node packing failed; loosen ctb")
        score = (rem - s).min(1).astype(np.float64) + 1e-3 * slots.sum(1)
        score[~cand] = -1e18
        t = int(np.argmax(score))
        rem[t] -= s
        slots[t, pr] -= 1
        tile_of[i] = t
    # slot within tile: parity-interleaved, stable placement order
    slot_of = np.empty(n, np.int64)
    for pr in (0, 1):
        m = node_par == pr
        key = tile_of[m].astype(np.int64)
        o = np.argsort(key, kind="stable")
        within = np.arange(m.sum()) - np.searchsorted(key[o], key[o])
        sl = np.empty(m.sum(), np.int64)
        sl[o] = key[o] * P + 2 * within + pr
        slot_of[m] = sl
    return slot_of


def prep_host(x, edge_index, cfg: Cfg):
    N, NC, NPC_RAW, NPC, TPC = (cfg.n_nodes, cfg.n_cores, cfg.npc_raw,
                                cfg.npc, cfg.tpc)
    SPT, NST, NB, CTB, CPG, CPT, J, GI, BS = (
        cfg.spt, cfg.nst, cfg.nb, cfg.ctb, cfg.cpg, cfg.cpt, cfg.j,
        cfg.gi, cfg.bs)
    src = np.asarray(edge_index[0], dtype=np.int64)
    dst = np.asarray(edge_index[1], dtype=np.int64)
    E = src.shape[0]

    deg = np.bincount(src, minlength=N)
    if deg.min() == 0:
        raise NotImplementedError("deg-0 nodes present; Wg+Ws fusion invalid")
    inv_deg = (1.0 / deg).astype(np.float32)

    c_src = np.minimum(src // NPC_RAW, NC - 1)
    c_dst = np.minimum(dst // NPC_RAW, NC - 1)
    # bucket = which HALF of the owner core's tiles dst lands in; preserved
    # statically by the permutation (like parity) so the per-half AllGather
    # can overlap with compute.
    bucket = ((dst >> 1) & 1).astype(np.int64)
    par = (dst % 2).astype(np.int64)       # original-id parity (preserved)

    # per-node out-degree per (bucket,parity), then pack nodes into tiles
    key4 = (bucket * 2 + par).astype(np.int64)
    cnt4 = np.zeros((N, 4), np.int32)
    np.add.at(cnt4, (src, key4), 1)
    perm_pos = np.zeros(N, np.int64)       # orig id -> slot within core
    for c in range(NC):
        lo, hi = c * NPC_RAW, min((c + 1) * NPC_RAW, N)
        ids = np.arange(lo, hi)
        perm_pos[lo:hi] = _pack_core(cnt4[lo:hi], (ids % 2).astype(np.int64),
                                     ((ids >> 1) & 1).astype(np.int64),
                                     TPC, CTB * P)
    gpos = np.minimum(np.arange(N) // NPC_RAW, NC - 1) * NPC + perm_pos

    # edge -> (core, supertile, chunk, partition) slots
    half_npc = NPC // 2
    pp_dst = perm_pos[dst]
    # bucket-A table is core-major [core][row]; bucket-B is PIECE-major
    # [piece][core][row] so the 3 partial AllGathers write contiguous
    # slices (pieces: within-half rows [0,3B), [3B,4B), [4B,5B),
    # B = SPT*P rows).
    stp = (NPC // TPC) * cfg.spt              # SPT*P rows
    b_plo = np.array([0, 3 * stp, 4 * stp], np.int64)
    b_phi = np.array([3 * stp, 4 * stp, 5 * stp], np.int64)
    b_base = np.concatenate([[0], np.cumsum(NC * (b_phi - b_plo))])[:3]
    r_half = pp_dst % half_npc
    piece = np.searchsorted(b_phi, r_half, side="right")
    rows_p = (b_phi - b_plo)[piece]
    rowB = b_base[piece] + np.minimum(dst // NPC_RAW, NC - 1) * rows_p \
        + (r_half - b_plo[piece])
    pairA = (np.minimum(dst // NPC_RAW, NC - 1) * (half_npc // 2)
             + r_half // 2)
    pair_in_bucket = np.where(bucket == 0, pairA, rowB >> 1)
    lsrc_val = (perm_pos[src] % P).astype(np.int16)
    tile_of_src = perm_pos[src] // P
    s_of = tile_of_src // SPT
    t_in = tile_of_src % SPT
    grp = ((((c_src * NST + s_of) * NB + bucket) * SPT + t_in)
           * 2 + par)
    n_groups = NC * NST * NB * SPT * 2
    counts = np.bincount(grp, minlength=n_groups)
    assert counts.max() <= CTB * P, (counts.max(), CTB * P)
    order = np.argsort(grp, kind="stable")
    starts = np.zeros(n_groups + 1, np.int64)
    np.cumsum(counts, out=starts[1:])
    q = np.arange(E) - starts[grp[order]]
    ks = grp[order]
    g_c = ks // (NST * NB * SPT * 2)
    g_s = (ks // (NB * SPT * 2)) % NST
    g_b = (ks // (SPT * 2)) % NB
    g_t = (ks // 2) % SPT
    g_p = ks % 2
    kk = q // P
    p_slot = q % P
    chunk = g_b * (SPT * CPG) + g_t * CPG + g_p * CTB + kk   # [0, J)
    c_loc = chunk - g_b * (SPT * CPG)                        # [0, J/NB)
    slot_i = c_loc * P + p_slot                              # [0, GI)

    idx_lin = np.zeros((NC, NST, NB, GI), np.int16)
    idx_lin[g_c, g_s, g_b, slot_i] = pair_in_bucket[order].astype(np.int16)
    # dma_gather is limited to 1024 indices per call on this runtime, so each
    # (supertile, bucket) region of GI slots is gathered by a sequence of
    # calls (cfg.calls).  Each call's index segment is independently wrapped:
    # wrapped[p, s16] = seg[s16*16 + p]; partitions replicated x8; calls and
    # buckets concatenated along the free dim.
    segs = []
    for b in range(NB):
        o = 0
        for (c0, nch) in cfg.calls:
            n = nch * P
            seg = idx_lin[..., b, o:o + n]                    # [NC,NST,n]
            segs.append(seg.reshape(NC, NST, n // 16, 16)
                        .transpose(0, 1, 3, 2))               # [NC,NST,16,n/16]
            o += n
        assert o == GI
    wrapped = np.concatenate(segs, axis=3)                    # [NC,NST,16,NB*GI/16]
    gidx16 = np.ascontiguousarray(np.tile(wrapped, (1, 1, 8, 1)))

    lsrc2 = np.full((NC, NST, P, J, 2), 300, np.int16)
    lsrc2[g_c, g_s, p_slot, chunk, 0] = lsrc_val[order]
    lsrc2[g_c, g_s, p_slot, chunk, 1] = lsrc_val[order]
    lsrc2 = np.ascontiguousarray(lsrc2.reshape(NC, NST, P, 2 * J))

    x = np.asarray(x, dtype=np.float32)
    x_pad = np.zeros((cfg.ntot, D), np.float32)
    invdeg_pad = np.zeros(cfg.ntot, np.float32)
    x_pad[gpos] = x
    invdeg_pad[gpos] = inv_deg
    x3 = x_pad.reshape(NC, 2, NPC // 2, D).astype(BF_NP)
    xfA = np.ascontiguousarray(x3[:, 0].reshape(cfg.ntot // 2, D))
    xfB = np.ascontiguousarray(np.concatenate(
        [x3[:, 1, lo:hi, :].reshape(-1, D)
         for lo, hi in zip(b_plo, b_phi)], axis=0))

    per_core = []
    for c in range(NC):
        xs = x_pad[c * NPC:(c + 1) * NPC]
        per_core.append(dict(
            x_hT=np.ascontiguousarray(xs.T.astype(BF_NP)),        # [64, NPC]
            xfA=xfA, xfB=xfB,                                     # [NTOT/2, 64]
            gidx16=np.ascontiguousarray(gidx16[c]),               # [NST,128,2*GI/16]
            lsrc2=np.ascontiguousarray(lsrc2[c]),                 # [NST,128,2J]
            invdegT=np.ascontiguousarray(
                invdeg_pad[c * NPC:(c + 1) * NPC].reshape(TPC, P).T),
        ))
    return per_core, perm_pos


def build_program(nc, cfg: Cfg):
    NPC, NTOT, NST, J, TPC, GI, NB, BS = (
        cfg.npc, cfg.ntot, cfg.nst, cfg.j, cfg.tpc, cfg.gi, cfg.nb, cfg.bs)
    NL = cfg.n_layers

    x_hT = nc.dram_tensor("x_hT", [D, NPC], BF16, kind="ExternalInput")
    xfA = nc.dram_tensor("xfA", [NTOT // 2, D], BF16, kind="ExternalInput")
    xfB = nc.dram_tensor("xfB", [NTOT // 2, D], BF16, kind="ExternalInput")
    gidx16 = nc.dram_tensor("gidx16", [NST, P, NB * (GI // 16)], I16,
                            kind="ExternalInput")
    lsrc2 = nc.dram_tensor("lsrc2", [NST, P, 2 * J], I16, kind="ExternalInput")
    invdegT = nc.dram_tensor("invdegT", [P, TPC], F32, kind="ExternalInput")
    w_gs = [nc.dram_tensor(f"w_gs{L}", [D, D], BF16, kind="ExternalInput")
            for L in range(NL)]
    w_l = [nc.dram_tensor(f"w_l{L}", [D, D], BF16, kind="ExternalInput")
           for L in range(NL)]
    w_c = [nc.dram_tensor(f"w_c{L}", [2, D], BF16, kind="ExternalInput")
           for L in range(NL)]
    out_own = nc.dram_tensor("out_own", [NPC, D], BF16, kind="ExternalOutput")

    ag_in = [nc.dram_tensor(f"ag_in{L}", [NPC, D], BF16, kind="Internal")
             for L in range(NL - 1)]
    shared = "Shared" if cfg.n_cores > 4 else "Local"
    h_fullA = [nc.dram_tensor(f"h_fullA{L}", [NTOT // 2, D], BF16,
                              kind="Internal", addr_space=shared)
               for L in range(NL - 1)]
    h_fullB = [nc.dram_tensor(f"h_fullB{L}", [NTOT // 2, D], BF16,
                              kind="Internal", addr_space=shared)
               for L in range(NL - 1)]

    with tile.TileContext(nc) as tc:
        _emit(nc, tc, cfg, locals())
    return nc


def _emit(nc, tc, cfg: Cfg, T):
    NPC, NTOT, NST, SPT, NB, CTB, CPG, J, TPC, GI, BS, NL = (
        cfg.npc, cfg.ntot, cfg.nst, cfg.spt, cfg.nb, cfg.ctb, cfg.cpg,
        cfg.j, cfg.tpc, cfg.gi, cfg.bs, cfg.n_layers)
    x_hT, xfA, xfB, gidx16, lsrc2, invdegT = (
        T["x_hT"], T["xfA"], T["xfB"], T["gidx16"], T["lsrc2"], T["invdegT"])
    w_gs, w_l, w_c, out_own = T["w_gs"], T["w_l"], T["w_c"], T["out_own"]
    ag_in, h_fullA, h_fullB = T["ag_in"], T["h_fullA"], T["h_fullB"]
    RCH = SPT * CPG               # chunks per bucket region within supertile
    # tile groups per supertile for the batched weight matmuls / ELU
    GROUPS = [(0, 4), (4, 4), (8, 2)]

    with (
        tc.tile_pool(name="const", bufs=1) as constp,
        tc.tile_pool(name="io", bufs=3) as iop,
        tc.tile_pool(name="bigX", bufs=3) as bigx,
        tc.tile_pool(name="bigS", bufs=3) as bigs,
        tc.tile_pool(name="small", bufs=3) as smallp,
        tc.tile_pool(name="grp", bufs=3) as grpp,
        tc.tile_pool(name="rows", bufs=2) as rowsp,
        tc.tile_pool(name="psA", bufs=3, space="PSUM") as psA,
        tc.tile_pool(name="psB", bufs=1, space="PSUM") as psB,
        tc.tile_pool(name="psC", bufs=2, space="PSUM") as psC,
        tc.tile_pool(name="psD", bufs=2, space="PSUM") as psD,
    ):
        identF = constp.tile([P, P], F32, name="identF")
        make_identity(nc, identF[:])
        iota16 = constp.tile([P, P], I16, name="iota16")
        nc.gpsimd.iota(iota16[:], pattern=[[1, P]], base=0,
                       channel_multiplier=0)
        invdeg_sb = constp.tile([P, TPC], F32, name="invdeg_sb")
        nc.sync.dma_start(invdeg_sb[:], invdegT[:])
        wgs_sb, wl_sb, wc_sb = [], [], []
        for L in range(NL):
            wg_t = constp.tile([D, D], BF16, name=f"wgs_sb{L}")
            nc.sync.dma_start(wg_t[:], w_gs[L][:])
            wgs_sb.append(wg_t)
            wl_t = constp.tile([D, D], BF16, name=f"wl_sb{L}")
            nc.sync.dma_start(wl_t[:], w_l[L][:])
            wl_sb.append(wl_t)
            wc_t = constp.tile([2, D], BF16, name=f"wc_sb{L}")
            nc.sync.dma_start(wc_t[:], w_c[L][:])
            wc_sb.append(wc_t)
        ones2 = constp.tile([2, SPT * P], BF16, name="ones2")
        nc.vector.memset(ones2[:], 1.0)
        negone = constp.tile([D, 1], F32, name="negone")
        nc.vector.memset(negone[:], -1.0)
        ones_f = constp.tile([D, 4 * P], F32, name="ones_f")
        nc.vector.memset(ones_f[:], 1.0)
        # layer-input h^T, SBUF-resident across the whole kernel
        hT_all = constp.tile([D, NPC], BF16, name="hT_all")
        nc.sync.dma_start(hT_all[:], x_hT[:])
        r_full = nc.gpsimd.to_reg(8 * P)
        r_regs = {8 * P: r_full}
        # per-queue DMA completion sems for PREPARE_ONLY gathers: desc-gen
        # runs ahead of data deps (which defer to the trigger), so the Q7
        # feeder never stalls on buffer/collective waits mid-stream.
        gat_sems = [nc.alloc_semaphore(f"gat_dma_q{q}") for q in range(4)]

        def emit_gather(L, s, X3, idx_t, tables, bsel):
            qn = (s * NB + bsel) * len(cfg.calls)
            for (c0, nch) in cfg.calls:
                col0 = bsel * (GI // 16) + c0 * (P // 16)
                n = nch * P
                if n not in r_regs:
                    r_regs[n] = nc.gpsimd.to_reg(n)
                nc.gpsimd.dma_gather(
                    X3[:, bsel * RCH + c0:bsel * RCH + c0 + nch, :],
                    tables[bsel],
                    idx_t[:, col0:col0 + nch * (P // 16)],
                    n, r_regs[n], P, elem_step=P,
                    single_packet=(n <= 1024),
                    queue_num=qn % 4)
                qn += 1

        for L in range(NL):
            last = L == NL - 1
            tabs = ((xfA, xfB) if L == 0
                    else (h_fullA[L - 1], h_fullB[L - 1]))
            tables = [t[:].rearrange("(a b) d -> a (b d)", b=2)
                      for t in tabs]                # [NPAIR/2, 128] views

            def compute_st(s, X3, ls2_t, L=L, last=last):
                ls3 = ls2_t[:].rearrange("p (c b) -> p c b", b=2)
                io3 = iota16[:].rearrange("p (a b) -> p a b", b=2)
                Sp = {}

                def s_build(pp):
                    # indicator chunks for tile pair pp (tiles 2pp, 2pp+1):
                    # pair tile [128, 2*CPT chunks, 128], one op per bucket
                    # block; all operands packed-last-dim -> DVE 2x mode
                    St = bigs.tile([P, 2 * cfg.cpt * P], BF16, tag="Sp",
                                   name=f"S_{L}_{s}_{pp}")
                    Sp[pp] = St[:].rearrange("p (c e) -> p c e", e=P)
                    S4 = St[:].rearrange("p (c a b) -> p c a b", a=64, b=2)
                    w = 2 * CPG
                    for b in range(NB):
                        lo = b * RCH + 2 * pp * CPG
                        nc.vector.tensor_tensor(
                            out=S4[:, b * w:(b + 1) * w],
                            in0=ls3[:, lo:lo + w, None, :]
                                .to_broadcast([P, w, 64, 2]),
                            in1=io3[:, None, :, :]
                                .to_broadcast([P, w, 64, 2]),
                            op=mybir.AluOpType.is_equal,
                        )

                rows_st = rowsp.tile([P, SPT * D], BF16, tag="rows",
                                     name=f"rw_{L}_{s}")
                nmPs = [None] * SPT
                outPs = {}
                nmTg = {}
                h32g = {}

                def scatter_pair(ta, tb):
                    pa = psA.tile([P, D], F32, tag="nmP",
                                  name=f"nmP_{L}_{s}_{ta}")
                    pb = psA.tile([P, D], F32, tag="nmP",
                                  name=f"nmP_{L}_{s}_{tb}")
                    nmPs[ta], nmPs[tb] = pa, pb
                    St = Sp.pop(ta // 2)
                    ci = 0
                    for b in range(NB):
                        for pr in range(2):
                            for k in range(CTB):
                                for tt, pp in ((0, pa), (1, pb)):
                                    ch = b * RCH + (ta + tt) * CPG \
                                        + pr * CTB + k
                                    sc = b * 2 * CPG + tt * CPG \
                                        + pr * CTB + k
                                    nc.tensor.matmul(
                                        pp[:],
                                        lhsT=St[:, sc, :],
                                        rhs=X3[:, ch, pr * D:(pr + 1) * D],
                                        start=(ci == 0),
                                        stop=(ci == cfg.cpt - 1),
                                    )
                                ci += 1

                nmTPs = {}

                def tail(t, g, gw):
                    g_t = s * SPT + t
                    nm_sb = smallp.tile([P, D], F32, tag="nm",
                                        name=f"nm_{L}_{s}_{t}")
                    nc.vector.tensor_scalar_mul(
                        nm_sb[:], nmPs[t][:], invdeg_sb[:, g_t:g_t + 1])
                    if t % 2 == 0:
                        nmTPs[t // 2] = psB.tile([D, 2 * P], F32, tag="nmTP",
                                                 name=f"nmTP_{L}_{s}_{t}")
                    nmTP = nmTPs[t // 2][:, (t % 2) * P:(t % 2 + 1) * P]
                    nc.tensor.transpose(nmTP, nm_sb[:], identF[:])
                    if g not in nmTg:
                        nmTg[g] = grpp.tile([D, 4 * P], BF16, tag="nmTg",
                                            name=f"nmTg_{L}_{s}_{g}")
                    off = (t - GROUPS[g][0]) * P
                    nc.scalar.copy(nmTg[g][:, off:off + P], nmTP)

                def group_ops(g):
                    t0, ntg = GROUPS[g]
                    W = ntg * P
                    c0 = s * SPT + t0
                    outP = psC.tile([D, 4 * P], F32, tag="outP",
                                    name=f"oP_{L}_{s}_{g}")
                    outPs[g] = outP
                    reg = outP[:, :W]
                    nc.tensor.matmul(reg, lhsT=wgs_sb[L][:],
                                     rhs=hT_all[:, c0 * P:c0 * P + W],
                                     start=True, stop=False)
                    nc.tensor.matmul(reg, lhsT=wl_sb[L][:],
                                     rhs=nmTg[g][:, :W],
                                     start=False, stop=False)
                    nc.tensor.matmul(reg, lhsT=wc_sb[L][:],
                                     rhs=ones2[:, :W],
                                     start=False, stop=True)
                    # ELU'+1: h' = max(z+1, min(exp(z), 1)); outP holds z+1
                    e_sb = smallp.tile([D, 4 * P], F32, tag="elu_e",
                                       name=f"e_{L}_{s}_{g}")
                    nc.scalar.activation(e_sb[:, :W], reg,
                                         mybir.ActivationFunctionType.Exp,
                                         bias=negone[:])
                    c_sb = smallp.tile([D, 4 * P], F32, tag="elu_c",
                                       name=f"c_{L}_{s}_{g}")
                    nc.vector.tensor_tensor(out=c_sb[:, :W], in0=e_sb[:, :W],
                                            in1=ones_f[:, :W],
                                            op=mybir.AluOpType.min)
                    h32 = grpp.tile([D, 4 * P], F32, tag="h32",
                                    name=f"h32_{L}_{s}_{g}")
                    h32g[g] = h32
                    nc.vector.tensor_tensor(out=h32[:, :W], in0=c_sb[:, :W],
                                            in1=reg, op=mybir.AluOpType.max)
                    if not last:
                        nc.scalar.copy(hT_all[:, c0 * P:c0 * P + W],
                                       h32[:, :W])
                    # row-major rows via PE transposes + one DVE evac
                    rPg = psD.tile([P, 4 * D], F32, tag="rPg",
                                   name=f"rPg_{L}_{s}_{g}")
                    for t in range(t0, t0 + ntg):
                        nc.tensor.transpose(
                            rPg[:, (t - t0) * D:(t - t0 + 1) * D],
                            h32[:, (t - t0) * P:(t - t0 + 1) * P],
                            identF[:D, :D])
                    if last:
                        nc.vector.tensor_scalar_add(
                            rows_st[:, t0 * D:(t0 + ntg) * D],
                            rPg[:, :ntg * D], -1.0)
                    else:
                        nc.vector.tensor_copy(
                            rows_st[:, t0 * D:(t0 + ntg) * D],
                            rPg[:, :ntg * D])

                grp_of_t = {}
                for g, (t0g, ntg) in enumerate(GROUPS):
                    for t in range(t0g, t0g + ntg):
                        grp_of_t[t] = g
                npair = SPT // 2
                s_build(0)
                for p in range(npair + 1):
                    if p < npair:
                        if p + 1 < npair:
                            s_build(p + 1)
                        scatter_pair(2 * p, 2 * p + 1)
                    if p >= 1:
                        for t in (2 * (p - 1), 2 * (p - 1) + 1):
                            g = grp_of_t[t]
                            tail(t, g, None)
                            if t == GROUPS[g][0] + GROUPS[g][1] - 1:
                                group_ops(g)

                dst_rows = (out_own if last else ag_in[L]).rearrange(
                    "(s t p) d -> s p t d", s=NST, t=SPT, p=P)
                nc.sync.dma_start(
                    dst_rows[s],
                    rows_st[:].rearrange("p (t d) -> p t d", d=D))

            def emit_ag(cs, L=L, last=last):
                # AllGathers fire as soon as the rows they need are written.
                # bucket-B table (piece-major layout) gathered in 3 slices so
                # only the last supertile's small tail is exposed at the
                # layer boundary.
                if last:
                    return
                stp = SPT * P
                if cs == NST // 2 - 1:
                    nc.gpsimd.collective_compute(
                        "AllGather",
                        mybir.AluOpType.bypass,
                        replica_groups=[list(range(cfg.n_cores))],
                        ins=[ag_in[L][0:NPC // 2, :]],
                        outs=[h_fullA[L][:]],
                    )
                if cs in (NST - 3, NST - 2):
                    r0 = 0 if cs == NST - 3 else 3 * stp
                    r1 = 3 * stp if cs == NST - 3 else 4 * stp
                    nc.gpsimd.collective_compute(
                        "AllGather",
                        mybir.AluOpType.bypass,
                        replica_groups=[list(range(cfg.n_cores))],
                        ins=[ag_in[L][NPC // 2 + r0:NPC // 2 + r1, :]],
                        outs=[h_fullB[L][cfg.n_cores * r0:
                                         cfg.n_cores * r1, :]],
                    )
                if cs == NST - 1:
                    nc.gpsimd.collective_compute(
                        "AllGather",
                        mybir.AluOpType.bypass,
                        replica_groups=[list(range(cfg.n_cores))],
                        ins=[ag_in[L][NPC - stp:NPC, :]],
                        outs=[h_fullB[L][cfg.n_cores * 4 * stp:, :]],
                    )

            for s in range(NST):
                idx_t = iop.tile([P, NB * (GI // 16)], I16, tag="idx",
                                 name=f"idx_{L}_{s}")
                nc.sync.dma_start(idx_t[:], gidx16[s, :, :])
                ls2_t = iop.tile([P, 2 * J], I16, tag="ls2",
                                 name=f"ls2_{L}_{s}")
                nc.sync.dma_start(ls2_t[:], lsrc2[s, :, :])
                X = bigx.tile([P, J * P], BF16, tag="X", name=f"X_{L}_{s}")
                X3 = X[:].rearrange("p (c e) -> p c e", e=P)
                emit_gather(L, s, X3, idx_t, tables, 0)
                emit_gather(L, s, X3, idx_t, tables, 1)
                compute_st(s, X3, ls2_t)
                emit_ag(s)


def kernel(**inputs):
    cfg = Cfg()
    x = np.asarray(inputs["x"], np.float32)
    ei = np.asarray(inputs["edge_index"])
    Ws = []
    for L, (a, b, c, bias) in enumerate(
            [("Wg1", "Wl1", "Ws1", "b1"), ("Wg2", "Wl2", "Ws2", "b2"),
             ("Wgo", "Wlo", "Wso", "bo")]):
        bv = np.asarray(inputs[bias], np.float32)
        assert np.all(bv == 0.0), "nonzero bias not supported by this build"
        wgs = (np.asarray(inputs[a], np.float32)
               + np.asarray(inputs[c], np.float32))
        wl = np.asarray(inputs[b], np.float32)
        # const row for the z+1 shift: layer 0 inputs are unshifted (raw x),
        # later layers consume h' = h+1, so subtract the weight column sums.
        cvec = np.ones(D, np.float32)
        if L > 0:
            cvec = cvec - wgs.sum(0) - wl.sum(0)
        c_hi = cvec.astype(BF_NP)
        c_lo = (cvec - c_hi.astype(np.float32)).astype(BF_NP)
        Ws.append((wgs.astype(BF_NP), wl.astype(BF_NP),
                   np.ascontiguousarray(np.stack([c_hi, c_lo]))))

    per_core, perm_pos = prep_host(x, ei, cfg)

    nc = bacc.Bacc("TRN2", target_bir_lowering=False, debug=False,
                   enable_asserts=False, num_devices=cfg.n_cores,
                   num_swdge_queues=4)
    build_program(nc, cfg)
    nc.compile()

    in_maps = []
    for c in range(cfg.n_cores):
        m = dict(per_core[c])
        for L in range(3):
            m[f"w_gs{L}"] = Ws[L][0]
            m[f"w_l{L}"] = Ws[L][1]
            m[f"w_c{L}"] = Ws[L][2]
        in_maps.append(m)

    res = run_bass_kernel_spmd(
        nc, in_maps, core_ids=list(range(cfg.n_cores)),
        trace=bool(int(os.environ.get("GNN_TRACE", "0"))),
    )
    full = np.zeros((cfg.n_nodes, D), np.float32)
    for c in range(cfg.n_cores):
        lo = c * cfg.npc_raw
        hi = min((c + 1) * cfg.npc_raw, cfg.n_nodes)
        full[lo:hi] = res.results[c]["out_own"].astype(
            np.float32)[perm_pos[lo:hi]]
    kernel.last_results = res
    return full.astype(np.float32)

